# revision 41
# baseline (speedup 1.0000x reference)
"""Trainium2 Bass kernel for nn_Block_27187142983954 (dense transformer block,
per-position head-mixing attention). Data-parallel over batch: 8 cores, one
batch element each. Self-contained: hardcodes all shapes.

Per-core plan (S=4096 positions, E=1024, H=16 heads, D=64):
  - qkv projection on TensorE: stationary = x feature-major tiles (from a
    host-pretransposed bf16 copy of x), moving = host-pretransposed weight
    columns; biases folded in as rank-1 (K=1) accumulating matmuls.
  - attention (per-position bilinear over heads) on VectorE in position-major
    layout with broadcast access patterns: bf16 tensor_tensor muls in 2x mode,
    partial reduction by halving-tree TT adds (2x) + final tensor_reduce (fp32).
  - softmax without max-subtraction (scores are O(1) by construction); the
    1/denominator is applied after the attn@v contraction (linearity).
  - v is computed with host-permuted weight rows so its features land in
    (d,g) order, which keeps every broadcast AP's innermost dim contiguous.
  - proj/ff matmuls on TensorE with PE-transposed activations as stationary.
  - LayerNorm stats on ScalarE via activation accum_out (Identity/Square);
    rsigma = exp(-0.5*ln(var+eps)) so softmax-exp and LN share one ACT table
    set; ln_g/ln_b of LN1 are folded into the ff weights on the host.
"""

import sys

sys.path.insert(0, "/opt/trn_rl_repo")

import numpy as np
import ml_dtypes

E, H, DQ, DV = 1024, 16, 64, 64
B, S = 8, 4096
EPS = 1e-5
NT = S // 128  # 32 position tiles per core
BF = ml_dtypes.bfloat16
F8 = ml_dtypes.float8_e4m3
W8SCALE = 32.0  # qkv weights are ~1/32; prescale into fp8's normal range

_CACHE = {}


def _patch_tail_drain():
    """walrus in this container rejects >1 sem wait on a CTRL (Drain)
    instruction; spread the TileContext tail-drain waits over wait-nops."""
    import concourse.tile as tile
    import bass_rust
    from concourse.vector_clock import ScopedClock

    if getattr(tile.TileContext, "_drain_patched", False):
        return

    def _drain_and_barrier(self, tick_clock, wait_clock):
        nc = self.nc
        drain_inst = nc.sync.drain()
        wait_clock.add_sem_waits(
            drain_inst.ins, ScopedClock({None: tick_clock.global_clock})
        )
        si = drain_inst.ins.sync_info
        waits = list(si.on_wait) if si is not None else []
        if len(waits) > 1:
            drain_inst.ins.sync_info = bass_rust.SyncInfo(on_wait=[], on_update=[])
            for w in waits:
                nop = nc.sync.nop()
                nop.ins.sync_info = bass_rust.SyncInfo(on_wait=[w], on_update=[])
        nc.all_engine_barrier()
        assert self.sems is not None
        popped = nc._tile_sem_poison_stack.pop()
        assert popped is self._sem_poison
        nc.clear_and_free_semaphores(list(self.sems.allocated().values()))
        nc.all_engine_barrier()

    tile.TileContext._drain_and_barrier = _drain_and_barrier
    tile.TileContext._drain_patched = True


def _split_excess_waits(nc, max_on_op=1, max_on_nop=1):
    """walrus in this container rejects >2 sem waits on compute instruction
    structs and >1 on DMA/CTRL structs. Hoist excess waits onto preceding
    same-engine NOPs."""
    import concourse.mybir as mybir
    import bass_rust

    narrow = {"DMACopy", "Drain", "NoOp", "Memset", "TriggeredCopy"}
    cnt = 0
    for bb in nc.m.functions[0].blocks:
        il = bb.instructions
        out = []
        for inst in il:
            cap = 1 if inst.opcode in narrow else max_on_op
            si = inst.sync_info
            waits = list(si.on_wait) if si is not None and si.on_wait else []
            if len(waits) > cap:
                n_extra = len(waits) - cap
                extra, keep = waits[:n_extra], waits[n_extra:]
                for i0 in range(0, len(extra), max_on_nop):
                    chunk = extra[i0 : i0 + max_on_nop]
                    nop = mybir.InstNoOp(name=f"waitnop-{cnt}", ins=[], outs=[])
                    cnt += 1
                    nop.engine = inst.engine
                    nop.sync_info = bass_rust.SyncInfo(on_wait=chunk, on_update=[])
                    out.append(nop)
                inst.sync_info = bass_rust.SyncInfo(
                    on_wait=keep,
                    on_update=list(si.on_update) if si.on_update else [],
                )
            out.append(inst)
        il[:] = out


def _build_program(trivial_affine: bool):
    import concourse.bass as bass
    import concourse.tile as tile
    import concourse.mybir as mybir
    from concourse.masks import make_identity

    _patch_tail_drain()

    f32 = mybir.dt.float32
    bf16 = mybir.dt.bfloat16
    fp8 = mybir.dt.float8e4
    ALU = mybir.AluOpType
    ACT = mybir.ActivationFunctionType
    DR = mybir.MatmulPerfMode.DoubleRow

    nc = bass.Bass("TRN2", target_bir_lowering=False, debug=False, num_devices=1)

    x_pm = nc.dram_tensor("x_pm", [S, E], f32, kind="ExternalInput").ap()
    x_bf_d = nc.dram_tensor("x_bf", [S, E], bf16, kind="ExternalInput").ap()
    xT = nc.dram_tensor("xT", [E, S], fp8, kind="ExternalInput").ap()
    wqkvT_d = nc.dram_tensor("wqkvT", [E, 3 * E], fp8, kind="ExternalInput").ap()
    projT_d = nc.dram_tensor("projT", [E, E], bf16, kind="ExternalInput").ap()
    ffw2T_d = nc.dram_tensor("ffw2T", [E, E], bf16, kind="ExternalInput").ap()
    bqkv_d = nc.dram_tensor("bqkv", [1, 3 * E], bf16, kind="ExternalInput").ap()
    bproj_d = nc.dram_tensor("bproj", [1, E], bf16, kind="ExternalInput").ap()
    bff2_d = nc.dram_tensor("bff2", [1, E], bf16, kind="ExternalInput").ap()
    if not trivial_affine:
        g_rep_d = nc.dram_tensor("g_rep", [128, E], f32, kind="ExternalInput").ap()
        b_rep_d = nc.dram_tensor("b_rep", [128, E], f32, kind="ExternalInput").ap()
    out_d = nc.dram_tensor("out", [S, E], f32, kind="ExternalOutput").ap()

    xT_r = xT.rearrange("(t p) s -> p t s", p=128)  # [128, 8, S]
    wqkv_r = wqkvT_d.rearrange("(t p) o -> p t o", p=128)
    proj_r = projT_d.rearrange("(t p) o -> p t o", p=128)
    ffw2_r = ffw2T_d.rearrange("(t p) o -> p t o", p=128)

    with tile.TileContext(nc) as tc:
        import contextlib

        ctx = contextlib.ExitStack()
        with ctx:
            fixed = ctx.enter_context(tc.tile_pool(name="fixed", bufs=1))
            work = ctx.enter_context(tc.tile_pool(name="work", bufs=3))
            work1 = ctx.enter_context(tc.tile_pool(name="work1", bufs=1))
            stats = ctx.enter_context(tc.tile_pool(name="stats", bufs=8))
            psq = ctx.enter_context(tc.tile_pool(name="psq", bufs=4, space="PSUM"))
            pst = ctx.enter_context(tc.tile_pool(name="pst", bufs=2, space="PSUM"))
            psb = ctx.enter_context(tc.tile_pool(name="psb", bufs=2, space="PSUM"))

            # ---- fixed tensors ----
            wqkv_sb = fixed.tile([128, 8, 3 * E], fp8)
            for t in range(8):
                nc.sync.dma_start(out=wqkv_sb[:, t, :], in_=wqkv_r[:, t, :])
            proj_sb = fixed.tile([128, 8, E], bf16)
            ffw2_sb = fixed.tile([128, 8, E], bf16)
            for t in range(8):
                nc.sync.dma_start(out=proj_sb[:, t, :], in_=proj_r[:, t, :])
                nc.sync.dma_start(out=ffw2_sb[:, t, :], in_=ffw2_r[:, t, :])
            bqkv_sb = fixed.tile([1, 3 * E], bf16)
            nc.sync.dma_start(out=bqkv_sb, in_=bqkv_d)
            bproj_sb = fixed.tile([1, E], bf16)
            nc.sync.dma_start(out=bproj_sb, in_=bproj_d)
            bff2_sb = fixed.tile([1, E], bf16)
            nc.sync.dma_start(out=bff2_sb, in_=bff2_d)
            if not trivial_affine:
                g_rep = fixed.tile([128, E], f32)
                nc.sync.dma_start(out=g_rep, in_=g_rep_d)
                b_rep = fixed.tile([128, E], f32)
                nc.sync.dma_start(out=b_rep, in_=b_rep_d)
            ones_row = fixed.tile([1, 128], bf16)
            nc.vector.memset(ones_row, 1.0)
            ident = fixed.tile([128, 128], bf16)
            make_identity(nc, ident)
            eps_sb = fixed.tile([128, 1], f32)
            nc.vector.memset(eps_sb, EPS)

            inv_n = 1.0 / float(E)

            def layer_norm(z, rs_out, mrs_out, scratch_bf):
                """Compute rsigma and -mu*rsigma of z [128, E] (fp32)."""
                s1 = stats.tile([128, 1], f32, tag="s1")
                s2 = stats.tile([128, 1], f32, tag="s2")
                nc.scalar.activation(scratch_bf, z, ACT.Identity, accum_out=s1)
                nc.scalar.activation(scratch_bf, z, ACT.Square, accum_out=s2)
                mu = stats.tile([128, 1], f32, tag="mu")
                nc.vector.tensor_scalar_mul(mu, s1, inv_n)
                mu2 = stats.tile([128, 1], f32, tag="mu2")
                nc.vector.tensor_tensor(mu2, mu, mu, ALU.mult)
                var = stats.tile([128, 1], f32, tag="var")
                nc.vector.scalar_tensor_tensor(
                    var, in0=s2, scalar=inv_n, in1=mu2, op0=ALU.mult, op1=ALU.subtract
                )
                lnv = stats.tile([128, 1], f32, tag="lnv")
                nc.scalar.activation(lnv, var, ACT.Ln, bias=eps_sb)
                nc.scalar.activation(rs_out, lnv, ACT.Exp, scale=-0.5)
                nc.vector.scalar_tensor_tensor(
                    mrs_out, in0=mu, scalar=-1.0, in1=rs_out, op0=ALU.mult, op1=ALU.mult
                )

            for t in range(NT):
                s0 = t * 128
                xp = work.tile([128, E], f32, tag="xp")
                nc.sync.dma_start(out=xp, in_=x_pm[s0 : s0 + 128, :])
                xbf = work.tile([128, E], bf16, tag="xbf")
                nc.sync.dma_start(out=xbf, in_=x_bf_d[s0 : s0 + 128, :])
                xf = work.tile([128, 8, 128], fp8, tag="xf")
                nc.sync.dma_start(out=xf, in_=xT_r[:, :, s0 : s0 + 128])

                # ---- qkv projection (fp8 DoubleRow, weights prescaled x32) ----
                qkv_sb = work1.tile([128, 3 * E], bf16, tag="qkv")
                for wave in range(2):
                    for j3 in range(3):
                        j = wave * 3 + j3
                        ps = psq.tile([128, 512], f32, tag="psq")
                        for e2 in range(4):
                            nc.tensor.matmul(
                                ps,
                                xf[:, 2 * e2 : 2 * e2 + 2, :],
                                wqkv_sb[:, 2 * e2 : 2 * e2 + 2, j * 512 : (j + 1) * 512],
                                start=(e2 == 0),
                                stop=False,
                                perf_mode=DR,
                            )
                        nc.tensor.matmul(
                            ps,
                            ones_row,
                            bqkv_sb[:, j * 512 : (j + 1) * 512],
                            start=False,
                            stop=True,
                        )
                        nc.scalar.mul(qkv_sb[:, j * 512 : (j + 1) * 512], ps, 1.0 / W8SCALE)

                q3 = qkv_sb[:, 0:E].rearrange("p (h d) -> p h d", h=H)
                k3 = qkv_sb[:, E : 2 * E].rearrange("p (g d) -> p g d", g=H)
                v3 = qkv_sb[:, 2 * E : 3 * E].rearrange("p (d g) -> p d g", d=DV)

                # ---- QK^T scores ----
                prod = work1.tile([128, 8, 16, 64], bf16, tag="prod")
                scr = work1.tile([128, 8192], bf16, tag="scr")
                scores = work.tile([128, H, H], f32, tag="scores")
                for half in range(2):
                    h0 = half * 8
                    qb = q3[:, h0 : h0 + 8, :].unsqueeze(2).broadcast_to([128, 8, 16, 64])
                    kb = k3.unsqueeze(1).broadcast_to([128, 8, 16, 64])
                    nc.vector.tensor_tensor(prod, kb, qb, ALU.mult)
                    t1 = scr[:, 0:4096].rearrange("p (a g d) -> p a g d", a=8, g=16)
                    nc.vector.tensor_tensor(
                        t1, prod[:, :, :, 0:32], prod[:, :, :, 32:64], ALU.add
                    )
                    t2 = scr[:, 4096:6144].rearrange("p (a g d) -> p a g d", a=8, g=16)
                    nc.vector.tensor_tensor(
                        t2, t1[:, :, :, 0:16], t1[:, :, :, 16:32], ALU.add
                    )
                    t3 = scr[:, 6144:7168].rearrange("p (a g d) -> p a g d", a=8, g=16)
                    nc.vector.tensor_tensor(
                        t3, t2[:, :, :, 0:8], t2[:, :, :, 8:16], ALU.add
                    )
                    t4 = scr[:, 7168:7680].rearrange("p (a g d) -> p a g d", a=8, g=16)
                    nc.vector.tensor_tensor(
                        t4, t3[:, :, :, 0:4], t3[:, :, :, 4:8], ALU.add
                    )
                    nc.vector.tensor_reduce(
                        scores[:, h0 : h0 + 8, :],
                        t4,
                        axis=mybir.AxisListType.X,
                        op=ALU.add,
                    )

                # ---- softmax (no max-subtraction; fold 1/den into p before AV) ----
                p_sb = work.tile([128, H, H], bf16, tag="p_sb")
                nc.scalar.activation(p_sb, scores, ACT.Exp)
                den = stats.tile([128, H], f32, tag="den")
                nc.vector.tensor_reduce(
                    den, p_sb, axis=mybir.AxisListType.X, op=ALU.add
                )
                rden = stats.tile([128, H], f32, tag="rden")
                nc.vector.reciprocal(rden, den)
                p_nm = work.tile([128, H, H], bf16, tag="p_nm")
                nc.vector.tensor_tensor(
                    p_nm,
                    p_sb,
                    rden.unsqueeze(2).broadcast_to([128, H, H]),
                    ALU.mult,
                )

                # ---- attn @ v ----
                attn_bf = work.tile([128, E], bf16, tag="attn_bf")
                a3 = attn_bf.rearrange("p (h d) -> p h d", h=H)
                prod_flat = prod.rearrange("p a g d -> p (a g d)")
                for half in range(2):
                    h0 = half * 8
                    # reuse prod's memory with a contiguous [128, 8, 64, 16] layout
                    pa = prod_flat.rearrange("p (a d g) -> p a d g", a=8, d=DV)
                    pb = (
                        p_nm[:, h0 : h0 + 8, :]
                        .unsqueeze(2)
                        .broadcast_to([128, 8, 64, 16])
                    )
                    vb = v3.unsqueeze(1).broadcast_to([128, 8, 64, 16])
                    nc.vector.tensor_tensor(pa, vb, pb, ALU.mult)
                    u1 = scr[:, 0:4096].rearrange("p (a d g) -> p a d g", a=8, d=64)
                    nc.vector.tensor_tensor(
                        u1, pa[:, :, :, 0:8], pa[:, :, :, 8:16], ALU.add
                    )
                    u2 = scr[:, 4096:6144].rearrange("p (a d g) -> p a d g", a=8, d=64)
                    nc.vector.tensor_tensor(
                        u2, u1[:, :, :, 0:4], u1[:, :, :, 4:8], ALU.add
                    )
                    u3 = scr[:, 6144:7168].rearrange("p (a d g) -> p a d g", a=8, d=64)
                    nc.vector.tensor_tensor(
                        u3, u2[:, :, :, 0:2], u2[:, :, :, 2:4], ALU.add
                    )
                    nc.vector.tensor_tensor(
                        a3[:, h0 : h0 + 8, :].unsqueeze(3),
                        u3[:, :, :, 0:1],
                        u3[:, :, :, 1:2],
                        ALU.add,
                    )

                # ---- transpose attn_out to feature-major ----
                attn_fm = work.tile([128, 8, 128], bf16, tag="attn_fm")
                for e in range(8):
                    pt = pst.tile([128, 128], bf16, tag="pst")
                    nc.tensor.transpose(pt, attn_bf[:, e * 128 : (e + 1) * 128], ident)
                    nc.vector.tensor_copy(attn_fm[:, e, :], pt)

                # ---- proj + residual (x folded in via identity matmul) ----
                z1 = work1.tile([128, E], f32, tag="z1")
                for j in range(2):
                    ps2 = psb.tile([128, 512], f32, tag="psb")
                    for e in range(8):
                        nc.tensor.matmul(
                            ps2,
                            attn_fm[:, e, :],
                            proj_sb[:, e, j * 512 : (j + 1) * 512],
                            start=(e == 0),
                            stop=False,
                        )
                    nc.tensor.matmul(
                        ps2,
                        ones_row,
                        bproj_sb[:, j * 512 : (j + 1) * 512],
                        start=False,
                        stop=False,
                    )
                    nc.tensor.matmul(
                        ps2,
                        ident,
                        xbf[:, j * 512 : (j + 1) * 512],
                        start=False,
                        stop=True,
                    )
                    nc.scalar.copy(z1[:, j * 512 : (j + 1) * 512], ps2)

                # ---- LN1 (g,b folded into ff weights) ----
                lnscr = work1.tile([128, E], bf16, tag="lnscr")
                rs1 = stats.tile([128, 1], f32, tag="rs1")
                mrs1 = stats.tile([128, 1], f32, tag="mrs1")
                layer_norm(z1, rs1, mrs1, lnscr)
                ln1_bf = work.tile([128, E], bf16, tag="ln1_bf")
                nc.scalar.activation(ln1_bf, z1, ACT.Identity, bias=mrs1, scale=rs1)

                ln1_fm = work.tile([128, 8, 128], bf16, tag="ln1_fm")
                for e in range(8):
                    pt = pst.tile([128, 128], bf16, tag="pst")
                    nc.tensor.transpose(pt, ln1_bf[:, e * 128 : (e + 1) * 128], ident)
                    nc.vector.tensor_copy(ln1_fm[:, e, :], pt)

                # ---- ff + gelu + residual ----
                z2 = work1.tile([128, E], f32, tag="z2")
                gl = work1.tile([128, E], f32, tag="gl")
                for j in range(2):
                    ps3 = psb.tile([128, 512], f32, tag="psb")
                    for e in range(8):
                        nc.tensor.matmul(
                            ps3,
                            ln1_fm[:, e, :],
                            ffw2_sb[:, e, j * 512 : (j + 1) * 512],
                            start=(e == 0),
                            stop=False,
                        )
                    nc.tensor.matmul(
                        ps3,
                        ones_row,
                        bff2_sb[:, j * 512 : (j + 1) * 512],
                        start=False,
                        stop=True,
                    )
                    nc.scalar.activation(
                        gl[:, j * 512 : (j + 1) * 512], ps3, ACT.Gelu
                    )
                    nc.vector.tensor_tensor(
                        z2[:, j * 512 : (j + 1) * 512],
                        gl[:, j * 512 : (j + 1) * 512],
                        xp[:, j * 512 : (j + 1) * 512],
                        ALU.add,
                    )

                # ---- LN2 + affine ----
                rs2 = stats.tile([128, 1], f32, tag="rs2")
                mrs2 = stats.tile([128, 1], f32, tag="mrs2")
                layer_norm(z2, rs2, mrs2, lnscr)
                if trivial_affine:
                    out_t = work.tile([128, E], f32, tag="out_t")
                    nc.scalar.activation(out_t, z2, ACT.Identity, bias=mrs2, scale=rs2)
                else:
                    zn = work1.tile([128, E], f32, tag="zn")
                    nc.scalar.activation(zn, z2, ACT.Identity, bias=mrs2, scale=rs2)
                    zn2 = work1.tile([128, E], f32, tag="zn2")
                    nc.gpsimd.tensor_tensor(zn2, zn, g_rep, ALU.mult)
                    out_t = work.tile([128, E], f32, tag="out_t")
                    nc.gpsimd.tensor_tensor(out_t, zn2, b_rep, ALU.add)
                nc.sync.dma_start(out=out_d[s0 : s0 + 128, :], in_=out_t)

    _split_excess_waits(nc)
    return nc


def _host_prep(inputs, trivial_affine=None):
    x = np.asarray(inputs["x"], np.float32)
    qk_w = np.asarray(inputs["qk_w"], np.float32)
    qk_b = np.asarray(inputs["qk_b"], np.float32)
    v_w = np.asarray(inputs["v_w"], np.float32)
    v_b = np.asarray(inputs["v_b"], np.float32)
    proj_w = np.asarray(inputs["proj_w"], np.float32)
    proj_b = np.asarray(inputs["proj_b"], np.float32)
    ff_w = np.asarray(inputs["ff_w"], np.float32)
    ff_b = np.asarray(inputs["ff_b"], np.float32)
    ln_g = np.asarray(inputs["ln_g"], np.float32)
    ln_b = np.asarray(inputs["ln_b"], np.float32)

    if trivial_affine is None:
        trivial_affine = bool(
            np.allclose(ln_g, 1.0, atol=1e-7) and np.allclose(ln_b, 0.0, atol=1e-7)
        )

    scale = 1.0 / np.sqrt(DQ).astype(np.float32)
    Wq = qk_w[:E] * scale
    bq = qk_b[:E] * scale
    Wk = qk_w[E:]
    bk = qk_b[E:]
    g_idx, d_idx = np.meshgrid(np.arange(H), np.arange(DV), indexing="ij")
    perm = np.empty(E, np.int64)
    perm[(d_idx * H + g_idx).ravel()] = (g_idx * DV + d_idx).ravel()
    Wv2 = v_w[perm]
    bv2 = v_b[perm]

    wqkvT = np.ascontiguousarray(
        (np.concatenate([Wq, Wk, Wv2], 0) * W8SCALE).T.astype(F8)
    )  # [E, 3E] fp8, prescaled
    bqkv = (np.concatenate([bq, bk, bv2]) * W8SCALE)[None, :].astype(BF)  # [1, 3E]
    projT = np.ascontiguousarray(proj_w.T.astype(BF))  # [E, E]
    bproj = proj_b[None, :].astype(BF)
    ffw2T = np.ascontiguousarray((ff_w * ln_g[None, :]).T.astype(BF))
    bff2 = (ff_b + ff_w @ ln_b)[None, :].astype(BF)

    shared = {
        "wqkvT": wqkvT,
        "bqkv": bqkv,
        "projT": projT,
        "bproj": bproj,
        "ffw2T": ffw2T,
        "bff2": bff2,
    }
    if not trivial_affine:
        shared["g_rep"] = np.ascontiguousarray(
            np.broadcast_to(ln_g[None, :], (128, E)), np.float32
        )
        shared["b_rep"] = np.ascontiguousarray(
            np.broadcast_to(ln_b[None, :], (128, E)), np.float32
        )
    in_maps = []
    for b in range(B):
        xb = np.ascontiguousarray(x[b])  # [S, E] f32
        xTb = np.ascontiguousarray(xb.T.astype(F8))  # [E, S] fp8
        m = {"x_pm": xb, "x_bf": xb.astype(BF), "xT": xTb}
        m.update(shared)
        in_maps.append(m)
    return in_maps


def kernel(**inputs) -> np.ndarray:
    from concourse.bass_utils import run_bass_kernel_spmd

    trivial_affine = bool(
        np.allclose(np.asarray(inputs["ln_g"]), 1.0, atol=1e-7)
        and np.allclose(np.asarray(inputs["ln_b"]), 0.0, atol=1e-7)
    )
    key = ("nc", trivial_affine)
    if key not in _CACHE:
        _CACHE[key] = _build_program(trivial_affine)
    nc = _CACHE[key]

    in_maps = _host_prep(inputs, trivial_affine)
    res = run_bass_kernel_spmd(nc, in_maps, core_ids=list(range(B)))
    out = np.stack([res.results[b]["out"] for b in range(B)], 0)
    return out.astype(np.float32)


if __name__ == "__main__":
    rng = np.random.default_rng(0)
    ins = {
        "x": rng.standard_normal((B, S, E), np.float32),
        "qk_w": rng.standard_normal((2 * E, E), np.float32) * 0.03,
        "qk_b": rng.standard_normal((2 * E,), np.float32) * 0.03,
        "v_w": rng.standard_normal((E, E), np.float32) * 0.03,
        "v_b": rng.standard_normal((E,), np.float32) * 0.03,
        "proj_w": rng.standard_normal((E, E), np.float32) * 0.03,
        "proj_b": rng.standard_normal((E,), np.float32) * 0.03,
        "ff_w": rng.standard_normal((E, E), np.float32) * 0.03,
        "ff_b": rng.standard_normal((E,), np.float32) * 0.03,
        "ln_g": np.ones((E,), np.float32),
        "ln_b": np.zeros((E,), np.float32),
    }
    o = kernel(**ins)
    print("ran", o.shape, o.dtype)

